# revision 22
# baseline (speedup 1.0000x reference)
"""Multi-head attention (B=4, S=2048, D=1024, H=16, dh=64, causal) on 8
Trainium2 NeuronCores.

Sharding: core (b, g) = batch b in 0..3, head-group g in 0..1 (8 heads each).
Each core computes attention for its 8 heads over its batch and a partial
output projection; the host sums the two head-group partials per batch and
adds the bias corrections (bo + bv @ Wo, since the V bias is not applied on
device -- attn rows sum to 1 so it folds into a constant row; the K bias is
dropped entirely because a per-query constant added to every score cancels
in softmax).

v2 layout (vs the f32r baseline):
  - all inputs bf16 (halves the startup DMA), no VONES / no CM mask DMA
  - weights + first X chunk DMA'd first so matmuls start ~6us in
  - qt/kt are 64-partition tiles (no zero-padding memsets)
  - exp runs on score PAIRS ([128, 2, 512] psum spanning 2 banks) to halve
    ACT instruction count; causal masking via tensor_paged_mask (2x DVE op,
    mask generated from an iota + per-partition index vectors)
  - diagonal blocks are column-restricted (skip fully-masked quarters) in
    the scores matmul, the exp, and the attn@V matmul
  - per-chunk software pipeline: projections of token chunk c+1 and the
    output projection of chunk c-1 are interleaved into the attention
    instruction stream of chunk c, so the tensor engine fills the bubbles
    the ACT(exp)-bound attention inner loop would otherwise leave
  - output stored bf16; host upcasts and sums the two head-group partials
"""

import sys
import types

import numpy as np
import ml_dtypes


def _install_ntff_shim():
    """antenv.axon_hooks is absent in this image; recreate it and register the
    ctypes NTFF profile hook like trn_boot would, so trace=True works."""
    if "antenv.axon_hooks" in sys.modules:
        return
    mod = types.ModuleType("antenv.axon_hooks")
    state = {"hook": None}
    mod.set_axon_ntff_profile_hook = lambda h: state.__setitem__("hook", h)
    mod.get_axon_ntff_profile_hook = lambda: state["hook"]
    sys.modules["antenv.axon_hooks"] = mod
    try:
        import antenv

        antenv.axon_hooks = mod
    except ImportError:
        pass
    try:
        from trn_agent_boot.trn_boot import _ntff_profile_via_ctypes

        mod.set_axon_ntff_profile_hook(
            _ntff_profile_via_ctypes("/opt/axon/libaxon_pjrt.so")
        )
    except Exception:
        pass


_install_ntff_shim()

import concourse.bacc as bacc  # noqa: E402
import concourse.mybir as mybir  # noqa: E402
import concourse.tile as tile  # noqa: E402

P = 128
D = 1024
FG = 512  # features per core = 8 heads x 64
NH = 8  # heads per core
DH = 64
KC = D // P  # 8 contraction chunks for the projections
FC = FG // P  # 4 feature chunks of 128
TQ = 512  # q tile (free dim)
TK = 128  # k tile (partition dim)
F32 = mybir.dt.float32
BF16 = mybir.dt.bfloat16
AF = mybir.ActivationFunctionType


def build(tokens=2048, causal=True):
    S = tokens
    NQC = S // TQ
    NKC = S // TK
    nc = bacc.Bacc()
    xt = nc.dram_tensor("XT", [D, S], BF16, kind="ExternalInput")
    wq = nc.dram_tensor("WQ", [D, FG], BF16, kind="ExternalInput")
    wk = nc.dram_tensor("WK", [D, FG], BF16, kind="ExternalInput")
    wv = nc.dram_tensor("WV", [D, FG], BF16, kind="ExternalInput")
    bq = nc.dram_tensor("BQ", [DH, NH], F32, kind="ExternalInput")
    wo = nc.dram_tensor("WO", [P, FC, D], BF16, kind="ExternalInput")
    cm = nc.dram_tensor("CM", [P, 4, TQ], BF16, kind="ExternalInput")
    out = nc.dram_tensor("OUT", [S, D], BF16, kind="ExternalOutput")

    with tile.TileContext(nc) as tc, nc.allow_low_precision(
        reason="bf16 matmul inputs"
    ):
        with tc.tile_pool(name="const", bufs=1) as cpool, tc.tile_pool(
            name="qkv", bufs=1
        ) as qkv, tc.tile_pool(name="w", bufs=1) as wpool, tc.tile_pool(
            name="xt", bufs=2
        ) as xpool, tc.tile_pool(name="e", bufs=6) as epool, tc.tile_pool(
            name="r", bufs=2
        ) as rpool, tc.tile_pool(name="o", bufs=3) as opool, tc.tile_pool(
            name="pss", bufs=2, space="PSUM"
        ) as pss, tc.tile_pool(
            name="pso", bufs=2, space="PSUM"
        ) as pso, tc.tile_pool(name="pj", bufs=2, space="PSUM") as pjp:
            # ---- DMAs split across the two hardware DGE rings (sync +
            # scalar); the first chunk's X and Wq are split per-kc so the
            # first projection matmuls can start as soon as each 128-row
            # slice lands (Tile tracks subtile deps) ----
            bq_sb = cpool.tile([DH, NH], F32, name="bq_sb")
            nc.sync.dma_start(bq_sb[:], bq[:])

            wq_sb = wpool.tile([P, KC, FG], BF16, name="wq_sb")
            wk_sb = wpool.tile([P, KC, FG], BF16, name="wk_sb")
            wv_sb = wpool.tile([P, KC, FG], BF16, name="wv_sb")

            xts = [None] * NQC

            def fetch_x(c):
                xts[c] = xpool.tile([P, KC, TQ], BF16, tag="xt", name="xt_t")
                if c == 0:
                    for kc in range(KC):
                        nc.sync.dma_start(
                            xts[c][:, kc, :],
                            xt[kc * P : (kc + 1) * P, 0:TQ],
                        )
                        nc.scalar.dma_start(
                            wq_sb[:, kc, :], wq[kc * P : (kc + 1) * P, :]
                        )
                else:
                    nc.sync.dma_start(
                        xts[c][:],
                        xt[:, c * TQ : (c + 1) * TQ].rearrange(
                            "(kc p) t -> p kc t", p=P
                        ),
                    )

            fetch_x(0)
            nc.scalar.dma_start(
                wk_sb[:], wk.rearrange("(kc p) m -> p kc m", p=P)
            )
            nc.scalar.dma_start(
                wv_sb[:], wv.rearrange("(kc p) m -> p kc m", p=P)
            )
            cm_sb = cpool.tile([P, 4, TQ], BF16, name="cm_sb")
            nc.scalar.dma_start(cm_sb[:], cm[:])

            qt_sb = qkv.tile([DH, NH, S], BF16, name="qt_sb")
            kt_sb = qkv.tile([DH, NH, S], BF16, name="kt_sb")
            v_sb = qkv.tile([P, NKC, NH, 2 * DH], BF16, name="v_sb")
            u_sb = qkv.tile([P, FC, S], BF16, name="u_sb")
            # ones block for the softmax-denominator rows of attn@V
            nc.vector.memset(v_sb[:, :, :, 0:DH], 1.0)

            wo_sb = wpool.tile([P, FC, D], BF16, name="wo_sb")

            # ---- projection emission for one token chunk, split in units ----
            # unit list interleaves Q/K/V so each is spread over the chunk
            UNITS = [
                ("q", 0), ("k", 0), ("v", 0),
                ("q", 1), ("k", 1), ("v", 1),
                ("q", 2), ("k", 2), ("v", 2),
                ("q", 3), ("k", 3), ("v", 3),
            ]

            def emit_unit(c, u):
                kind, idx = UNITS[u]
                tsl = slice(c * TQ, (c + 1) * TQ)
                if kind in ("q", "k"):
                    w_sb = wq_sb if kind == "q" else wk_sb
                    dst = qt_sb if kind == "q" else kt_sb
                    ps = pjp.tile([P, TQ], F32, tag="pj", name="ps_qk")
                    for kc in range(KC):
                        nc.tensor.matmul(
                            ps[:],
                            w_sb[:, kc, idx * P : (idx + 1) * P],
                            xts[c][:, kc, :],
                            start=(kc == 0),
                            stop=(kc == KC - 1),
                        )
                    for hh in range(2):
                        hx = 2 * idx + hh
                        if kind == "q":
                            nc.vector.tensor_tensor(
                                dst[:, hx, tsl],
                                ps[hh * DH : (hh + 1) * DH, :],
                                bq_sb[:, hx : hx + 1].to_broadcast([DH, TQ]),
                                mybir.AluOpType.add,
                            )
                        else:
                            nc.vector.tensor_copy(
                                dst[:, hx, tsl], ps[hh * DH : (hh + 1) * DH, :]
                            )
                else:
                    ps = pjp.tile([P, NH, DH], F32, tag="pj", name="ps_v")
                    for kc in range(KC):
                        nc.tensor.matmul(
                            ps[:],
                            xts[c][:, kc, idx * P : (idx + 1) * P],
                            wv_sb[:, kc, :],
                            start=(kc == 0),
                            stop=(kc == KC - 1),
                        )
                    tc_idx = c * (TQ // P) + idx
                    nc.vector.tensor_copy(v_sb[:, tc_idx, :, DH : 2 * DH], ps[:])

            # ---- attention for one (head, q-chunk) ----
            def emit_av(po, h, e_t, pr, kind, stop):
                kc0 = 2 * pr
                c0s = (256, 384) if kind == 2 else (0, 128) if kind == 1 else (0, 0)
                for s in range(2):
                    c0 = c0s[s]
                    nc.tensor.matmul(
                        po[:, c0:],
                        v_sb[:, kc0 + s, h, :],
                        e_t[:, s, c0:],
                        start=(pr == 0 and s == 0),
                        stop=(stop and s == 1),
                        skip_group_check=True,
                    )

            def attn(h, qc):
                hi, hp = h // 2, h % 2
                qtsl = slice(qc * TQ, (qc + 1) * TQ)
                npairs = (2 * (qc + 1)) if causal else (NKC // 2)
                po = pso.tile([P, TQ], F32, tag="po", name="po")
                pend = None
                for pr in range(npairs):
                    kc0 = 2 * pr
                    kind = 0
                    if causal:
                        if kc0 == 4 * qc:
                            kind = 1
                        elif kc0 == 4 * qc + 2:
                            kind = 2
                    ps = pss.tile([P, 2, TQ], F32, tag="ps", name="ps_s")
                    e_t = epool.tile([P, 2, TQ], BF16, tag="e", name="e_t")
                    c0 = 256 if kind == 2 else 0
                    for s in range(2):
                        nc.tensor.matmul(
                            ps[:, s, c0:],
                            kt_sb[:, h, (kc0 + s) * TK : (kc0 + s + 1) * TK],
                            qt_sb[:, h, qc * TQ + c0 : (qc + 1) * TQ],
                            start=True,
                            stop=True,
                        )
                    nc.scalar.activation(
                        e_t[:, :, c0:], ps[:, :, c0:], AF.Exp, scale=0.125
                    )
                    if kind:
                        # mask each kc block; only columns the attn@V matmul
                        # will actually read need masking
                        mc0s = (256, 384) if kind == 2 else (0, 128)
                        for s in range(2):
                            mc = mc0s[s]
                            j = 2 * (kind - 1) + s
                            nc.vector.tensor_mul(
                                e_t[:, s, mc:],
                                e_t[:, s, mc:],
                                cm_sb[:, j, mc:],
                            )
                    if pend is not None:
                        emit_av(po, h, *pend, stop=False)
                    pend = (e_t, pr, kind)
                emit_av(po, h, *pend, stop=True)
                # rows 0-63 of po hold Z replicated (ones block in v_sb)
                rb = rpool.tile([DH, TQ], F32, tag="rb", name="rb_t")
                nc.vector.reciprocal_approx_fast(rb[:], po[0:DH, :])
                nc.vector.tensor_mul(
                    u_sb[hp * DH : (hp + 1) * DH, hi, qtsl], po[DH:P, :], rb[:]
                )

            # ---- output projection for one 128-token block ----
            def outproj_slice(qc, t8, evict_on_scalar=False):
                tg = qc * (TQ // P) + t8
                o_t = opool.tile([P, D], BF16, tag="o", name="o_t")
                for n2 in range(2):
                    ps = pjp.tile([P, 512], F32, tag="pj", name="ps_o")
                    for i in range(FC):
                        nc.tensor.matmul(
                            ps[:],
                            u_sb[:, i, tg * P : (tg + 1) * P],
                            wo_sb[:, i, n2 * 512 : (n2 + 1) * 512],
                            start=(i == 0),
                            stop=(i == FC - 1),
                        )
                    osl = o_t[:, n2 * 512 : (n2 + 1) * 512]
                    if evict_on_scalar:
                        # the scalar engine is idle after the last exp
                        nc.scalar.activation(osl, ps[:], AF.Copy)
                    else:
                        nc.vector.tensor_copy(osl, ps[:])
                nc.sync.dma_start(out[tg * P : (tg + 1) * P, :], o_t[:])

            # ---- schedule ----
            # warm the PE clock (HAM ramps to 2.4GHz after ~4us of continuous
            # matmul activity) with dummy matmuls on a scratch tile while the
            # input DMAs land; they write alternating psum banks, never read
            warm_sb = cpool.tile([P, TQ], BF16, name="warm_sb")
            nc.vector.memset(warm_sb[:], 0.0)
            for _ in range(34):
                wps = pjp.tile([P, TQ], F32, tag="pj", name="ps_warm")
                nc.tensor.matmul(
                    wps[:], warm_sb[:, 0:P], warm_sb[:], start=True, stop=True
                )
            # chunk 0 runs Q then K then V units, matching DMA arrival order
            for kind, idx in [("q", i) for i in range(4)] + [
                ("k", i) for i in range(4)
            ] + [("v", i) for i in range(4)]:
                emit_unit(0, UNITS.index((kind, idx)))
            for qc in range(NQC):
                if qc + 1 < NQC:
                    fetch_x(qc + 1)
                if qc == 0:
                    nc.sync.dma_start(wo_sb[:], wo[:])
                nunits = len(UNITS) if qc + 1 < NQC else 0
                udone = 0
                for h in range(NH):
                    attn(h, qc)
                    uend = nunits * (h + 1) // NH
                    while udone < uend:
                        emit_unit(qc + 1, udone)
                        udone += 1
                    if qc >= 1 and h % 2 == 1:
                        outproj_slice(qc - 1, h // 2)
                if qc == NQC - 1:
                    for t8 in range(TQ // P):
                        outproj_slice(qc, t8, evict_on_scalar=True)

    nc.compile()
    return nc


def make_in_maps(X, Wq, bq, Wk, Wv, Wo, causal):
    bf = ml_dtypes.bfloat16
    # mask[p, j, f] = 1.0 where k-position p of diagonal chunk j may attend
    # to q-position f of the 512-wide q tile: p <= f - 128*j
    pv = np.arange(P)[:, None, None]
    jv = np.arange(4)[None, :, None]
    fv = np.arange(TQ)[None, None, :]
    cmv = (pv <= fv - TK * jv).astype(bf)
    in_maps = []
    for b in range(4):
        for g in range(2):
            sl = slice(g * FG, (g + 1) * FG)
            in_maps.append(
                {
                    "XT": np.ascontiguousarray(X[b].T).astype(bf),
                    "WQ": np.ascontiguousarray(Wq[:, sl]).astype(bf),
                    "WK": np.ascontiguousarray(Wk[:, sl]).astype(bf),
                    "WV": np.ascontiguousarray(Wv[:, sl]).astype(bf),
                    "BQ": np.ascontiguousarray(bq[sl].reshape(NH, DH).T).astype(
                        np.float32
                    ),
                    "WO": np.ascontiguousarray(
                        Wo[sl, :].reshape(FC, P, D).transpose(1, 0, 2)
                    ).astype(bf),
                    "CM": cmv,
                }
            )
    return in_maps


_CACHE = {}


def _get_program(causal):
    key = bool(causal)
    if key not in _CACHE:
        _CACHE[key] = build(tokens=2048, causal=key)
    return _CACHE[key]


def kernel(X, Wq, bq, Wk, bk, Wv, bv, Wo, bo, causal, **_unused):
    from concourse.bass_utils import run_bass_kernel_spmd

    X = np.asarray(X, np.float32)
    Wq, bq = np.asarray(Wq, np.float32), np.asarray(bq, np.float32)
    Wk = np.asarray(Wk, np.float32)
    Wv = np.asarray(Wv, np.float32)
    Wo, bo = np.asarray(Wo, np.float32), np.asarray(bo, np.float32)
    bv = np.asarray(bv, np.float32)
    causal_flag = bool(np.asarray(causal).item())

    nc = _get_program(causal_flag)
    in_maps = make_in_maps(X, Wq, bq, Wk, Wv, Wo, causal_flag)
    res = run_bass_kernel_spmd(nc, in_maps, core_ids=list(range(8)))

    # attn rows sum to 1, so the missing V bias contributes bv @ Wo exactly
    corr = bv @ Wo + bo
    outs = []
    for b in range(4):
        o = (
            res.results[2 * b]["OUT"].astype(np.float32)
            + res.results[2 * b + 1]["OUT"].astype(np.float32)
            + corr
        )
        outs.append(o)
    return np.stack(outs).astype(np.float32)


# revision 25
# speedup vs baseline: 1.0360x; 1.0360x over previous
"""Multi-head attention (B=4, S=2048, D=1024, H=16, dh=64, causal) on 8
Trainium2 NeuronCores.

Sharding: core (b, g) = batch b in 0..3, head-group g in 0..1 (8 heads each).
Each core computes attention for its 8 heads over its batch and a partial
output projection; the host sums the two head-group partials per batch and
adds the bias corrections (bo + bv @ Wo, since the V bias is not applied on
device -- attn rows sum to 1 so it folds into a constant row; the K bias is
dropped entirely because a per-query constant added to every score cancels
in softmax).

v2 layout (vs the f32r baseline):
  - all inputs bf16 (halves the startup DMA), no VONES / no CM mask DMA
  - weights + first X chunk DMA'd first so matmuls start ~6us in
  - qt/kt are 64-partition tiles (no zero-padding memsets)
  - exp runs on score PAIRS ([128, 2, 512] psum spanning 2 banks) to halve
    ACT instruction count; causal masking via tensor_paged_mask (2x DVE op,
    mask generated from an iota + per-partition index vectors)
  - diagonal blocks are column-restricted (skip fully-masked quarters) in
    the scores matmul, the exp, and the attn@V matmul
  - per-chunk software pipeline: projections of token chunk c+1 and the
    output projection of chunk c-1 are interleaved into the attention
    instruction stream of chunk c, so the tensor engine fills the bubbles
    the ACT(exp)-bound attention inner loop would otherwise leave
  - output stored bf16; host upcasts and sums the two head-group partials
"""

import sys
import types

import numpy as np
import ml_dtypes


def _install_ntff_shim():
    """antenv.axon_hooks is absent in this image; recreate it and register the
    ctypes NTFF profile hook like trn_boot would, so trace=True works."""
    if "antenv.axon_hooks" in sys.modules:
        return
    mod = types.ModuleType("antenv.axon_hooks")
    state = {"hook": None}
    mod.set_axon_ntff_profile_hook = lambda h: state.__setitem__("hook", h)
    mod.get_axon_ntff_profile_hook = lambda: state["hook"]
    sys.modules["antenv.axon_hooks"] = mod
    try:
        import antenv

        antenv.axon_hooks = mod
    except ImportError:
        pass
    try:
        from trn_agent_boot.trn_boot import _ntff_profile_via_ctypes

        mod.set_axon_ntff_profile_hook(
            _ntff_profile_via_ctypes("/opt/axon/libaxon_pjrt.so")
        )
    except Exception:
        pass


_install_ntff_shim()

import concourse.bacc as bacc  # noqa: E402
import concourse.mybir as mybir  # noqa: E402
import concourse.tile as tile  # noqa: E402

P = 128
D = 1024
FG = 512  # features per core = 8 heads x 64
NH = 8  # heads per core
DH = 64
KC = D // P  # 8 contraction chunks for the projections
FC = FG // P  # 4 feature chunks of 128
TQ = 512  # q tile (free dim)
TK = 128  # k tile (partition dim)
F32 = mybir.dt.float32
BF16 = mybir.dt.bfloat16
AF = mybir.ActivationFunctionType


def build(tokens=2048, causal=True):
    S = tokens
    NQC = S // TQ
    NKC = S // TK
    nc = bacc.Bacc()
    xt = nc.dram_tensor("XT", [D, S], BF16, kind="ExternalInput")
    wq = nc.dram_tensor("WQ", [D, FG], BF16, kind="ExternalInput")
    wk = nc.dram_tensor("WK", [D, FG], BF16, kind="ExternalInput")
    wv = nc.dram_tensor("WV", [D, FG], BF16, kind="ExternalInput")
    bq = nc.dram_tensor("BQ", [DH, NH], F32, kind="ExternalInput")
    wo = nc.dram_tensor("WO", [P, FC, D], BF16, kind="ExternalInput")
    cm = nc.dram_tensor("CM", [P, 4, TQ], BF16, kind="ExternalInput")
    out = nc.dram_tensor("OUT", [S, D], BF16, kind="ExternalOutput")

    with tile.TileContext(nc) as tc, nc.allow_low_precision(
        reason="bf16 matmul inputs"
    ):
        with tc.tile_pool(name="const", bufs=1) as cpool, tc.tile_pool(
            name="qkv", bufs=1
        ) as qkv, tc.tile_pool(name="w", bufs=1) as wpool, tc.tile_pool(
            name="xt", bufs=2
        ) as xpool, tc.tile_pool(name="e", bufs=6) as epool, tc.tile_pool(
            name="r", bufs=2
        ) as rpool, tc.tile_pool(name="o", bufs=3) as opool, tc.tile_pool(
            name="pss", bufs=2, space="PSUM"
        ) as pss, tc.tile_pool(
            name="pso", bufs=2, space="PSUM"
        ) as pso, tc.tile_pool(name="pj", bufs=2, space="PSUM") as pjp:
            # ---- DMAs split across the two hardware DGE rings (sync +
            # scalar); the first chunk's X and Wq are split per-kc so the
            # first projection matmuls can start as soon as each 128-row
            # slice lands (Tile tracks subtile deps) ----
            bq_sb = cpool.tile([DH, NH], F32, name="bq_sb")
            nc.sync.dma_start(bq_sb[:], bq[:])

            wq_sb = wpool.tile([P, KC, FG], BF16, name="wq_sb")
            wk_sb = wpool.tile([P, KC, FG], BF16, name="wk_sb")
            wv_sb = wpool.tile([P, KC, FG], BF16, name="wv_sb")

            xts = [None] * NQC

            def fetch_x(c):
                xts[c] = xpool.tile([P, KC, TQ], BF16, tag="xt", name="xt_t")
                if c == 0:
                    for kc in range(KC):
                        nc.sync.dma_start(
                            xts[c][:, kc, :],
                            xt[kc * P : (kc + 1) * P, 0:TQ],
                        )
                        nc.scalar.dma_start(
                            wq_sb[:, kc, :], wq[kc * P : (kc + 1) * P, :]
                        )
                else:
                    nc.sync.dma_start(
                        xts[c][:],
                        xt[:, c * TQ : (c + 1) * TQ].rearrange(
                            "(kc p) t -> p kc t", p=P
                        ),
                    )

            fetch_x(0)
            nc.scalar.dma_start(
                wk_sb[:], wk.rearrange("(kc p) m -> p kc m", p=P)
            )
            nc.scalar.dma_start(
                wv_sb[:], wv.rearrange("(kc p) m -> p kc m", p=P)
            )
            cm_sb = cpool.tile([P, 4, TQ], BF16, name="cm_sb")
            nc.scalar.dma_start(cm_sb[:], cm[:])

            qt_sb = qkv.tile([DH, NH, S], BF16, name="qt_sb")
            kt_sb = qkv.tile([DH, NH, S], BF16, name="kt_sb")
            v_sb = qkv.tile([P, NKC, NH, 2 * DH], BF16, name="v_sb")
            u_sb = qkv.tile([P, FC, S], BF16, name="u_sb")

            wo_sb = wpool.tile([P, FC, D], BF16, name="wo_sb")

            # ---- projection emission for one token chunk, split in units ----
            # unit list interleaves Q/K/V so each is spread over the chunk
            UNITS = [
                ("q", 0), ("k", 0), ("v", 0),
                ("q", 1), ("k", 1), ("v", 1),
                ("q", 2), ("k", 2), ("v", 2),
                ("q", 3), ("k", 3), ("v", 3),
            ]

            def emit_unit(c, u):
                kind, idx = UNITS[u]
                tsl = slice(c * TQ, (c + 1) * TQ)
                if kind in ("q", "k"):
                    w_sb = wq_sb if kind == "q" else wk_sb
                    dst = qt_sb if kind == "q" else kt_sb
                    ps = pjp.tile([P, TQ], F32, tag="pj", name="ps_qk")
                    for kc in range(KC):
                        nc.tensor.matmul(
                            ps[:],
                            w_sb[:, kc, idx * P : (idx + 1) * P],
                            xts[c][:, kc, :],
                            start=(kc == 0),
                            stop=(kc == KC - 1),
                        )
                    for hh in range(2):
                        hx = 2 * idx + hh
                        if kind == "q":
                            nc.vector.tensor_tensor(
                                dst[:, hx, tsl],
                                ps[hh * DH : (hh + 1) * DH, :],
                                bq_sb[:, hx : hx + 1].to_broadcast([DH, TQ]),
                                mybir.AluOpType.add,
                            )
                        else:
                            nc.vector.tensor_copy(
                                dst[:, hx, tsl], ps[hh * DH : (hh + 1) * DH, :]
                            )
                else:
                    ps = pjp.tile([P, NH, DH], F32, tag="pj", name="ps_v")
                    for kc in range(KC):
                        nc.tensor.matmul(
                            ps[:],
                            xts[c][:, kc, idx * P : (idx + 1) * P],
                            wv_sb[:, kc, :],
                            start=(kc == 0),
                            stop=(kc == KC - 1),
                        )
                    tc_idx = c * (TQ // P) + idx
                    nc.vector.tensor_copy(v_sb[:, tc_idx, :, DH : 2 * DH], ps[:])

            # ---- attention for one (head, q-chunk) ----
            def emit_av(po, h, e_t, pr, kind, stop):
                kc0 = 2 * pr
                c0s = (256, 384) if kind == 2 else (0, 128) if kind == 1 else (0, 0)
                for s in range(2):
                    c0 = c0s[s]
                    nc.tensor.matmul(
                        po[:, c0:],
                        v_sb[:, kc0 + s, h, :],
                        e_t[:, s, c0:],
                        start=(pr == 0 and s == 0),
                        stop=(stop and s == 1),
                        skip_group_check=True,
                    )

            def attn(h, qc):
                hi, hp = h // 2, h % 2
                qtsl = slice(qc * TQ, (qc + 1) * TQ)
                npairs = (2 * (qc + 1)) if causal else (NKC // 2)
                po = pso.tile([P, TQ], F32, tag="po", name="po")
                pend = None
                for pr in range(npairs):
                    kc0 = 2 * pr
                    kind = 0
                    if causal:
                        if kc0 == 4 * qc:
                            kind = 1
                        elif kc0 == 4 * qc + 2:
                            kind = 2
                    ps = pss.tile([P, 2, TQ], F32, tag="ps", name="ps_s")
                    e_t = epool.tile([P, 2, TQ], BF16, tag="e", name="e_t")
                    c0 = 256 if kind == 2 else 0
                    for s in range(2):
                        nc.tensor.matmul(
                            ps[:, s, c0:],
                            kt_sb[:, h, (kc0 + s) * TK : (kc0 + s + 1) * TK],
                            qt_sb[:, h, qc * TQ + c0 : (qc + 1) * TQ],
                            start=True,
                            stop=True,
                        )
                    nc.scalar.activation(
                        e_t[:, :, c0:], ps[:, :, c0:], AF.Exp, scale=0.125
                    )
                    if kind:
                        # mask each kc block; only columns the attn@V matmul
                        # will actually read need masking
                        mc0s = (256, 384) if kind == 2 else (0, 128)
                        for s in range(2):
                            mc = mc0s[s]
                            j = 2 * (kind - 1) + s
                            nc.vector.tensor_mul(
                                e_t[:, s, mc:],
                                e_t[:, s, mc:],
                                cm_sb[:, j, mc:],
                            )
                    if pend is not None:
                        emit_av(po, h, *pend, stop=False)
                    pend = (e_t, pr, kind)
                emit_av(po, h, *pend, stop=True)
                # rows 0-63 of po hold Z replicated (ones block in v_sb)
                rb = rpool.tile([DH, TQ], F32, tag="rb", name="rb_t")
                nc.vector.reciprocal_approx_fast(rb[:], po[0:DH, :])
                nc.vector.tensor_mul(
                    u_sb[hp * DH : (hp + 1) * DH, hi, qtsl], po[DH:P, :], rb[:]
                )

            # ---- output projection for one 128-token block ----
            def outproj_slice(qc, t8, evict_on_scalar=False):
                tg = qc * (TQ // P) + t8
                o_t = opool.tile([P, D], BF16, tag="o", name="o_t")
                for n2 in range(2):
                    ps = pjp.tile([P, 512], F32, tag="pj", name="ps_o")
                    for i in range(FC):
                        nc.tensor.matmul(
                            ps[:],
                            u_sb[:, i, tg * P : (tg + 1) * P],
                            wo_sb[:, i, n2 * 512 : (n2 + 1) * 512],
                            start=(i == 0),
                            stop=(i == FC - 1),
                        )
                    osl = o_t[:, n2 * 512 : (n2 + 1) * 512]
                    if evict_on_scalar:
                        # the scalar engine is idle after the last exp
                        nc.scalar.activation(osl, ps[:], AF.Copy)
                    else:
                        nc.vector.tensor_copy(osl, ps[:])
                nc.sync.dma_start(out[tg * P : (tg + 1) * P, :], o_t[:])

            # ---- schedule ----
            # warm the PE clock (HAM ramps to 2.4GHz after ~4us of continuous
            # matmul activity) with dummy matmuls on a scratch tile while the
            # input DMAs land; they write alternating psum banks, never read.
            # both memsets go on the otherwise-idle gpsimd engine so the
            # vector queue stays clear for projection evictions
            warm_sb = cpool.tile([P, TQ], BF16, name="warm_sb")
            nc.gpsimd.memset(warm_sb[:], 0.0)
            # ones block for the softmax-denominator rows of attn@V
            nc.gpsimd.memset(v_sb[:, :, :, 0:DH], 1.0)
            for _ in range(34):
                wps = pjp.tile([P, TQ], F32, tag="pj", name="ps_warm")
                nc.tensor.matmul(
                    wps[:], warm_sb[:, 0:P], warm_sb[:], start=True, stop=True
                )
            # chunk 0 runs Q then K then V units, matching DMA arrival order
            for kind, idx in [("q", i) for i in range(4)] + [
                ("k", i) for i in range(4)
            ] + [("v", i) for i in range(4)]:
                emit_unit(0, UNITS.index((kind, idx)))
            for qc in range(NQC):
                if qc + 1 < NQC:
                    fetch_x(qc + 1)
                if qc == 0:
                    nc.sync.dma_start(wo_sb[:], wo[:])
                nunits = len(UNITS) if qc + 1 < NQC else 0
                udone = 0
                for h in range(NH):
                    attn(h, qc)
                    uend = nunits * (h + 1) // NH
                    while udone < uend:
                        emit_unit(qc + 1, udone)
                        udone += 1
                    # output projections trail their chunk by two windows so
                    # this filler work lands in the ACT-bound final window
                    if qc == 2 and h % 2 == 1:
                        outproj_slice(0, h // 2)
                    elif qc == 3 and h % 2 == 1:
                        outproj_slice(1, h // 2)
                    elif qc == 3 and h % 2 == 0:
                        outproj_slice(2, h // 2)
                if qc == NQC - 1:
                    for t8 in range(TQ // P):
                        outproj_slice(qc, t8, evict_on_scalar=True)

    nc.compile()
    return nc


def make_in_maps(X, Wq, bq, Wk, Wv, Wo, causal):
    bf = ml_dtypes.bfloat16
    # mask[p, j, f] = 1.0 where k-position p of diagonal chunk j may attend
    # to q-position f of the 512-wide q tile: p <= f - 128*j
    pv = np.arange(P)[:, None, None]
    jv = np.arange(4)[None, :, None]
    fv = np.arange(TQ)[None, None, :]
    cmv = (pv <= fv - TK * jv).astype(bf)
    in_maps = []
    for b in range(4):
        for g in range(2):
            sl = slice(g * FG, (g + 1) * FG)
            in_maps.append(
                {
                    "XT": np.ascontiguousarray(X[b].T).astype(bf),
                    "WQ": np.ascontiguousarray(Wq[:, sl]).astype(bf),
                    "WK": np.ascontiguousarray(Wk[:, sl]).astype(bf),
                    "WV": np.ascontiguousarray(Wv[:, sl]).astype(bf),
                    "BQ": np.ascontiguousarray(bq[sl].reshape(NH, DH).T).astype(
                        np.float32
                    ),
                    "WO": np.ascontiguousarray(
                        Wo[sl, :].reshape(FC, P, D).transpose(1, 0, 2)
                    ).astype(bf),
                    "CM": cmv,
                }
            )
    return in_maps


_CACHE = {}


def _get_program(causal):
    key = bool(causal)
    if key not in _CACHE:
        _CACHE[key] = build(tokens=2048, causal=key)
    return _CACHE[key]


def kernel(X, Wq, bq, Wk, bk, Wv, bv, Wo, bo, causal, **_unused):
    from concourse.bass_utils import run_bass_kernel_spmd

    X = np.asarray(X, np.float32)
    Wq, bq = np.asarray(Wq, np.float32), np.asarray(bq, np.float32)
    Wk = np.asarray(Wk, np.float32)
    Wv = np.asarray(Wv, np.float32)
    Wo, bo = np.asarray(Wo, np.float32), np.asarray(bo, np.float32)
    bv = np.asarray(bv, np.float32)
    causal_flag = bool(np.asarray(causal).item())

    nc = _get_program(causal_flag)
    in_maps = make_in_maps(X, Wq, bq, Wk, Wv, Wo, causal_flag)
    res = run_bass_kernel_spmd(nc, in_maps, core_ids=list(range(8)))

    # attn rows sum to 1, so the missing V bias contributes bv @ Wo exactly
    corr = bv @ Wo + bo
    outs = []
    for b in range(4):
        o = (
            res.results[2 * b]["OUT"].astype(np.float32)
            + res.results[2 * b + 1]["OUT"].astype(np.float32)
            + corr
        )
        outs.append(o)
    return np.stack(outs).astype(np.float32)


# revision 28
# speedup vs baseline: 1.0417x; 1.0055x over previous
"""Multi-head attention (B=4, S=2048, D=1024, H=16, dh=64, causal) on 8
Trainium2 NeuronCores.

Sharding: core (b, g) = batch b in 0..3, head-group g in 0..1 (8 heads each).
Each core computes attention for its 8 heads over its batch and a partial
output projection; the host sums the two head-group partials per batch and
adds the bias corrections (bo + bv @ Wo, since the V bias is not applied on
device -- attn rows sum to 1 so it folds into a constant row; the K bias is
dropped entirely because a per-query constant added to every score cancels
in softmax).

v2 layout (vs the f32r baseline):
  - all inputs bf16 (halves the startup DMA), no VONES / no CM mask DMA
  - weights + first X chunk DMA'd first so matmuls start ~6us in
  - qt/kt are 64-partition tiles (no zero-padding memsets)
  - exp runs on score PAIRS ([128, 2, 512] psum spanning 2 banks) to halve
    ACT instruction count; causal masking via tensor_paged_mask (2x DVE op,
    mask generated from an iota + per-partition index vectors)
  - diagonal blocks are column-restricted (skip fully-masked quarters) in
    the scores matmul, the exp, and the attn@V matmul
  - per-chunk software pipeline: projections of token chunk c+1 and the
    output projection of chunk c-1 are interleaved into the attention
    instruction stream of chunk c, so the tensor engine fills the bubbles
    the ACT(exp)-bound attention inner loop would otherwise leave
  - output stored bf16; host upcasts and sums the two head-group partials
"""

import sys
import types

import numpy as np
import ml_dtypes


def _install_ntff_shim():
    """antenv.axon_hooks is absent in this image; recreate it and register the
    ctypes NTFF profile hook like trn_boot would, so trace=True works."""
    if "antenv.axon_hooks" in sys.modules:
        return
    mod = types.ModuleType("antenv.axon_hooks")
    state = {"hook": None}
    mod.set_axon_ntff_profile_hook = lambda h: state.__setitem__("hook", h)
    mod.get_axon_ntff_profile_hook = lambda: state["hook"]
    sys.modules["antenv.axon_hooks"] = mod
    try:
        import antenv

        antenv.axon_hooks = mod
    except ImportError:
        pass
    try:
        from trn_agent_boot.trn_boot import _ntff_profile_via_ctypes

        mod.set_axon_ntff_profile_hook(
            _ntff_profile_via_ctypes("/opt/axon/libaxon_pjrt.so")
        )
    except Exception:
        pass


_install_ntff_shim()

import concourse.bacc as bacc  # noqa: E402
import concourse.mybir as mybir  # noqa: E402
import concourse.tile as tile  # noqa: E402

P = 128
D = 1024
FG = 512  # features per core = 8 heads x 64
NH = 8  # heads per core
DH = 64
KC = D // P  # 8 contraction chunks for the projections
FC = FG // P  # 4 feature chunks of 128
TQ = 512  # q tile (free dim)
TK = 128  # k tile (partition dim)
F32 = mybir.dt.float32
BF16 = mybir.dt.bfloat16
AF = mybir.ActivationFunctionType


def build(tokens=2048, causal=True):
    S = tokens
    NQC = S // TQ
    NKC = S // TK
    nc = bacc.Bacc()
    xt = nc.dram_tensor("XT", [D, S], BF16, kind="ExternalInput")
    wq = nc.dram_tensor("WQ", [D, FG], BF16, kind="ExternalInput")
    wk = nc.dram_tensor("WK", [D, FG], BF16, kind="ExternalInput")
    wv = nc.dram_tensor("WV", [D, FG], BF16, kind="ExternalInput")
    bq = nc.dram_tensor("BQ", [DH, NH], F32, kind="ExternalInput")
    wo = nc.dram_tensor("WO", [P, FC, D], BF16, kind="ExternalInput")
    cm = nc.dram_tensor("CM", [P, 4, TQ], BF16, kind="ExternalInput")
    out = nc.dram_tensor("OUT", [S, D], BF16, kind="ExternalOutput")

    with tile.TileContext(nc) as tc, nc.allow_low_precision(
        reason="bf16 matmul inputs"
    ):
        with tc.tile_pool(name="const", bufs=1) as cpool, tc.tile_pool(
            name="qkv", bufs=1
        ) as qkv, tc.tile_pool(name="w", bufs=1) as wpool, tc.tile_pool(
            name="xt", bufs=2
        ) as xpool, tc.tile_pool(name="e", bufs=6) as epool, tc.tile_pool(
            name="r", bufs=2
        ) as rpool, tc.tile_pool(name="o", bufs=3) as opool, tc.tile_pool(
            name="pss", bufs=2, space="PSUM"
        ) as pss, tc.tile_pool(
            name="pso", bufs=2, space="PSUM"
        ) as pso, tc.tile_pool(name="pj", bufs=2, space="PSUM") as pjp:
            # ---- DMAs split across the two hardware DGE rings (sync +
            # scalar); the first chunk's X and Wq are split per-kc so the
            # first projection matmuls can start as soon as each 128-row
            # slice lands (Tile tracks subtile deps) ----
            bq_sb = cpool.tile([DH, NH], F32, name="bq_sb")
            nc.sync.dma_start(bq_sb[:], bq[:])

            wq_sb = wpool.tile([P, KC, FG], BF16, name="wq_sb")
            wk_sb = wpool.tile([P, KC, FG], BF16, name="wk_sb")
            wv_sb = wpool.tile([P, KC, FG], BF16, name="wv_sb")

            xts = [None] * NQC

            def fetch_x(c):
                xts[c] = xpool.tile([P, KC, TQ], BF16, tag="xt", name="xt_t")
                if c == 0:
                    for kc in range(KC):
                        nc.sync.dma_start(
                            xts[c][:, kc, :],
                            xt[kc * P : (kc + 1) * P, 0:TQ],
                        )
                        nc.scalar.dma_start(
                            wq_sb[:, kc, :], wq[kc * P : (kc + 1) * P, :]
                        )
                else:
                    nc.sync.dma_start(
                        xts[c][:],
                        xt[:, c * TQ : (c + 1) * TQ].rearrange(
                            "(kc p) t -> p kc t", p=P
                        ),
                    )

            fetch_x(0)
            nc.scalar.dma_start(
                wk_sb[:], wk.rearrange("(kc p) m -> p kc m", p=P)
            )
            nc.sync.dma_start(
                wv_sb[:], wv.rearrange("(kc p) m -> p kc m", p=P)
            )
            cm_sb = cpool.tile([P, 4, TQ], BF16, name="cm_sb")
            nc.scalar.dma_start(cm_sb[:], cm[:])

            qt_sb = qkv.tile([DH, NH, S], BF16, name="qt_sb")
            kt_sb = qkv.tile([DH, NH, S], BF16, name="kt_sb")
            v_sb = qkv.tile([P, NKC, NH, 2 * DH], BF16, name="v_sb")
            u_sb = qkv.tile([P, FC, S], BF16, name="u_sb")

            wo_sb = wpool.tile([P, FC, D], BF16, name="wo_sb")

            # ---- projection emission for one token chunk, split in units ----
            # unit list interleaves Q/K/V so each is spread over the chunk
            UNITS = [
                ("q", 0), ("k", 0), ("v", 0),
                ("q", 1), ("k", 1), ("v", 1),
                ("q", 2), ("k", 2), ("v", 2),
                ("q", 3), ("k", 3), ("v", 3),
            ]

            def emit_unit(c, u):
                kind, idx = UNITS[u]
                tsl = slice(c * TQ, (c + 1) * TQ)
                if kind in ("q", "k"):
                    w_sb = wq_sb if kind == "q" else wk_sb
                    dst = qt_sb if kind == "q" else kt_sb
                    ps = pjp.tile([P, TQ], F32, tag="pj", name="ps_qk")
                    for kc in range(KC):
                        nc.tensor.matmul(
                            ps[:],
                            w_sb[:, kc, idx * P : (idx + 1) * P],
                            xts[c][:, kc, :],
                            start=(kc == 0),
                            stop=(kc == KC - 1),
                        )
                    for hh in range(2):
                        hx = 2 * idx + hh
                        if kind == "q":
                            nc.vector.tensor_tensor(
                                dst[:, hx, tsl],
                                ps[hh * DH : (hh + 1) * DH, :],
                                bq_sb[:, hx : hx + 1].to_broadcast([DH, TQ]),
                                mybir.AluOpType.add,
                            )
                        else:
                            nc.vector.tensor_copy(
                                dst[:, hx, tsl], ps[hh * DH : (hh + 1) * DH, :]
                            )
                else:
                    ps = pjp.tile([P, NH, DH], F32, tag="pj", name="ps_v")
                    for kc in range(KC):
                        nc.tensor.matmul(
                            ps[:],
                            xts[c][:, kc, idx * P : (idx + 1) * P],
                            wv_sb[:, kc, :],
                            start=(kc == 0),
                            stop=(kc == KC - 1),
                        )
                    tc_idx = c * (TQ // P) + idx
                    nc.vector.tensor_copy(v_sb[:, tc_idx, :, DH : 2 * DH], ps[:])

            # ---- attention for one (head, q-chunk) ----
            def emit_av(po, h, e_t, pr, kind, stop):
                kc0 = 2 * pr
                c0s = (256, 384) if kind == 2 else (0, 128) if kind == 1 else (0, 0)
                for s in range(2):
                    c0 = c0s[s]
                    nc.tensor.matmul(
                        po[:, c0:],
                        v_sb[:, kc0 + s, h, :],
                        e_t[:, s, c0:],
                        start=(pr == 0 and s == 0),
                        stop=(stop and s == 1),
                        skip_group_check=True,
                    )

            def attn(h, qc):
                hi, hp = h // 2, h % 2
                qtsl = slice(qc * TQ, (qc + 1) * TQ)
                npairs = (2 * (qc + 1)) if causal else (NKC // 2)
                po = pso.tile([P, TQ], F32, tag="po", name="po")
                pend = None
                for pr in range(npairs):
                    kc0 = 2 * pr
                    kind = 0
                    if causal:
                        if kc0 == 4 * qc:
                            kind = 1
                        elif kc0 == 4 * qc + 2:
                            kind = 2
                    ps = pss.tile([P, 2, TQ], F32, tag="ps", name="ps_s")
                    e_t = epool.tile([P, 2, TQ], BF16, tag="e", name="e_t")
                    c0 = 256 if kind == 2 else 0
                    for s in range(2):
                        nc.tensor.matmul(
                            ps[:, s, c0:],
                            kt_sb[:, h, (kc0 + s) * TK : (kc0 + s + 1) * TK],
                            qt_sb[:, h, qc * TQ + c0 : (qc + 1) * TQ],
                            start=True,
                            stop=True,
                        )
                    nc.scalar.activation(
                        e_t[:, :, c0:], ps[:, :, c0:], AF.Exp, scale=0.125
                    )
                    if kind:
                        # mask each kc block; only columns the attn@V matmul
                        # will actually read need masking
                        mc0s = (256, 384) if kind == 2 else (0, 128)
                        for s in range(2):
                            mc = mc0s[s]
                            j = 2 * (kind - 1) + s
                            nc.vector.tensor_mul(
                                e_t[:, s, mc:],
                                e_t[:, s, mc:],
                                cm_sb[:, j, mc:],
                            )
                    if pend is not None:
                        emit_av(po, h, *pend, stop=False)
                    pend = (e_t, pr, kind)
                emit_av(po, h, *pend, stop=True)
                # rows 0-63 of po hold Z replicated (ones block in v_sb)
                rb = rpool.tile([DH, TQ], F32, tag="rb", name="rb_t")
                nc.vector.reciprocal_approx_fast(rb[:], po[0:DH, :])
                nc.vector.tensor_mul(
                    u_sb[hp * DH : (hp + 1) * DH, hi, qtsl], po[DH:P, :], rb[:]
                )

            # ---- output projection for one 128-token block ----
            def outproj_slice(qc, t8, final=False):
                tg = qc * (TQ // P) + t8
                o_t = opool.tile([P, D], BF16, tag="o", name="o_t")
                for n2 in range(2):
                    ps = pjp.tile([P, 512], F32, tag="pj", name="ps_o")
                    for i in range(FC):
                        nc.tensor.matmul(
                            ps[:],
                            u_sb[:, i, tg * P : (tg + 1) * P],
                            wo_sb[:, i, n2 * 512 : (n2 + 1) * 512],
                            start=(i == 0),
                            stop=(i == FC - 1),
                        )
                    osl = o_t[:, n2 * 512 : (n2 + 1) * 512]
                    if final and n2 == 0:
                        # split the drain-critical evictions across the
                        # then-idle scalar engine and the vector engine
                        nc.scalar.activation(osl, ps[:], AF.Copy)
                    else:
                        nc.vector.tensor_copy(osl, ps[:])
                eng = nc.scalar if final else nc.sync
                eng.dma_start(out[tg * P : (tg + 1) * P, :], o_t[:])

            # ---- schedule ----
            # warm the PE clock (HAM ramps to 2.4GHz after ~4us of continuous
            # matmul activity) with dummy matmuls on a scratch tile while the
            # input DMAs land; they write alternating psum banks, never read.
            # both memsets go on the otherwise-idle gpsimd engine so the
            # vector queue stays clear for projection evictions
            warm_sb = cpool.tile([P, TQ], BF16, name="warm_sb")
            nc.gpsimd.memset(warm_sb[:], 0.0)
            # ones block for the softmax-denominator rows of attn@V
            nc.gpsimd.memset(v_sb[:, :, :, 0:DH], 1.0)
            for _ in range(34):
                wps = pjp.tile([P, TQ], F32, tag="pj", name="ps_warm")
                nc.tensor.matmul(
                    wps[:], warm_sb[:, 0:P], warm_sb[:], start=True, stop=True
                )
            # chunk 0 runs Q then K then V units, matching DMA arrival order
            for kind, idx in [("q", i) for i in range(4)] + [
                ("k", i) for i in range(4)
            ] + [("v", i) for i in range(4)]:
                emit_unit(0, UNITS.index((kind, idx)))
            for qc in range(NQC):
                if qc + 1 < NQC:
                    fetch_x(qc + 1)
                if qc == 0:
                    nc.sync.dma_start(wo_sb[:], wo[:])
                nunits = len(UNITS) if qc + 1 < NQC else 0
                udone = 0
                for h in range(NH):
                    attn(h, qc)
                    uend = nunits * (h + 1) // NH
                    while udone < uend:
                        emit_unit(qc + 1, udone)
                        udone += 1
                    # output projections trail their chunk by two windows so
                    # this filler work lands in the ACT-bound final window
                    if qc == 2 and h % 2 == 1:
                        outproj_slice(0, h // 2)
                    elif qc == 3 and h % 2 == 1:
                        outproj_slice(1, h // 2)
                    elif qc == 3 and h % 2 == 0:
                        outproj_slice(2, h // 2)
                if qc == NQC - 1:
                    for t8 in range(TQ // P):
                        outproj_slice(qc, t8, final=True)

    nc.compile()
    return nc


def make_in_maps(X, Wq, bq, Wk, Wv, Wo, causal):
    bf = ml_dtypes.bfloat16
    # mask[p, j, f] = 1.0 where k-position p of diagonal chunk j may attend
    # to q-position f of the 512-wide q tile: p <= f - 128*j
    pv = np.arange(P)[:, None, None]
    jv = np.arange(4)[None, :, None]
    fv = np.arange(TQ)[None, None, :]
    cmv = (pv <= fv - TK * jv).astype(bf)
    in_maps = []
    for b in range(4):
        for g in range(2):
            sl = slice(g * FG, (g + 1) * FG)
            in_maps.append(
                {
                    "XT": np.ascontiguousarray(X[b].T).astype(bf),
                    "WQ": np.ascontiguousarray(Wq[:, sl]).astype(bf),
                    "WK": np.ascontiguousarray(Wk[:, sl]).astype(bf),
                    "WV": np.ascontiguousarray(Wv[:, sl]).astype(bf),
                    "BQ": np.ascontiguousarray(bq[sl].reshape(NH, DH).T).astype(
                        np.float32
                    ),
                    "WO": np.ascontiguousarray(
                        Wo[sl, :].reshape(FC, P, D).transpose(1, 0, 2)
                    ).astype(bf),
                    "CM": cmv,
                }
            )
    return in_maps


_CACHE = {}


def _get_program(causal):
    key = bool(causal)
    if key not in _CACHE:
        _CACHE[key] = build(tokens=2048, causal=key)
    return _CACHE[key]


def kernel(X, Wq, bq, Wk, bk, Wv, bv, Wo, bo, causal, **_unused):
    from concourse.bass_utils import run_bass_kernel_spmd

    X = np.asarray(X, np.float32)
    Wq, bq = np.asarray(Wq, np.float32), np.asarray(bq, np.float32)
    Wk = np.asarray(Wk, np.float32)
    Wv = np.asarray(Wv, np.float32)
    Wo, bo = np.asarray(Wo, np.float32), np.asarray(bo, np.float32)
    bv = np.asarray(bv, np.float32)
    causal_flag = bool(np.asarray(causal).item())

    nc = _get_program(causal_flag)
    in_maps = make_in_maps(X, Wq, bq, Wk, Wv, Wo, causal_flag)
    res = run_bass_kernel_spmd(nc, in_maps, core_ids=list(range(8)))

    # attn rows sum to 1, so the missing V bias contributes bv @ Wo exactly
    corr = bv @ Wo + bo
    outs = []
    for b in range(4):
        o = (
            res.results[2 * b]["OUT"].astype(np.float32)
            + res.results[2 * b + 1]["OUT"].astype(np.float32)
            + corr
        )
        outs.append(o)
    return np.stack(outs).astype(np.float32)


# revision 33
# speedup vs baseline: 1.0532x; 1.0110x over previous
"""Multi-head attention (B=4, S=2048, D=1024, H=16, dh=64, causal) on 8
Trainium2 NeuronCores.

Sharding: core (b, g) = batch b in 0..3, head-group g in 0..1 (8 heads each).
Each core computes attention for its 8 heads over its batch and a partial
output projection; the host sums the two head-group partials per batch and
adds the bias corrections (bo + bv @ Wo, since the V bias is not applied on
device -- attn rows sum to 1 so it folds into a constant row; the K bias is
dropped entirely because a per-query constant added to every score cancels
in softmax).

v2 layout (vs the f32r baseline):
  - all inputs bf16 (halves the startup DMA), no VONES / no CM mask DMA
  - weights + first X chunk DMA'd first so matmuls start ~6us in
  - qt/kt are 64-partition tiles (no zero-padding memsets)
  - exp runs on score PAIRS ([128, 2, 512] psum spanning 2 banks) to halve
    ACT instruction count; causal masking via a DMA'd 0/1 mask multiply
    restricted to the columns the attn@V matmul actually reads
  - diagonal blocks are column-restricted (skip fully-masked quarters) in
    the scores matmul, the exp, and the attn@V matmul
  - per-chunk software pipeline: projections of token chunk c+1 and the
    output projection of chunk c-1 are interleaved into the attention
    instruction stream of chunk c, so the tensor engine fills the bubbles
    the ACT(exp)-bound attention inner loop would otherwise leave
  - output stored bf16; host upcasts and sums the two head-group partials
"""

import sys
import types

import numpy as np
import ml_dtypes


def _install_ntff_shim():
    """antenv.axon_hooks is absent in this image; recreate it and register the
    ctypes NTFF profile hook like trn_boot would, so trace=True works."""
    if "antenv.axon_hooks" in sys.modules:
        return
    mod = types.ModuleType("antenv.axon_hooks")
    state = {"hook": None}
    mod.set_axon_ntff_profile_hook = lambda h: state.__setitem__("hook", h)
    mod.get_axon_ntff_profile_hook = lambda: state["hook"]
    sys.modules["antenv.axon_hooks"] = mod
    try:
        import antenv

        antenv.axon_hooks = mod
    except ImportError:
        pass
    try:
        from trn_agent_boot.trn_boot import _ntff_profile_via_ctypes

        mod.set_axon_ntff_profile_hook(
            _ntff_profile_via_ctypes("/opt/axon/libaxon_pjrt.so")
        )
    except Exception:
        pass


_install_ntff_shim()

import concourse.bacc as bacc  # noqa: E402
import concourse.mybir as mybir  # noqa: E402
import concourse.tile as tile  # noqa: E402

P = 128
D = 1024
FG = 512  # features per core = 8 heads x 64
NH = 8  # heads per core
DH = 64
KC = D // P  # 8 contraction chunks for the projections
FC = FG // P  # 4 feature chunks of 128
TQ = 512  # q tile (free dim)
TK = 128  # k tile (partition dim)
F32 = mybir.dt.float32
BF16 = mybir.dt.bfloat16
AF = mybir.ActivationFunctionType


def build(tokens=2048, causal=True):
    S = tokens
    NQC = S // TQ
    NKC = S // TK
    nc = bacc.Bacc()
    xt = nc.dram_tensor("XT", [D, S], BF16, kind="ExternalInput")
    wq = nc.dram_tensor("WQ", [D, FG], BF16, kind="ExternalInput")
    wk = nc.dram_tensor("WK", [D, FG], BF16, kind="ExternalInput")
    wv = nc.dram_tensor("WV", [D, FG], BF16, kind="ExternalInput")
    bq = nc.dram_tensor("BQ", [DH, NH], F32, kind="ExternalInput")
    wo = nc.dram_tensor("WO", [P, FC, D], BF16, kind="ExternalInput")
    cm = nc.dram_tensor("CM", [P, 4, TQ], BF16, kind="ExternalInput")
    out = nc.dram_tensor("OUT", [S, D], BF16, kind="ExternalOutput")

    with tile.TileContext(nc) as tc, nc.allow_low_precision(
        reason="bf16 matmul inputs"
    ):
        with tc.tile_pool(name="const", bufs=1) as cpool, tc.tile_pool(
            name="qkv", bufs=1
        ) as qkv, tc.tile_pool(name="w", bufs=1) as wpool, tc.tile_pool(
            name="xt", bufs=2
        ) as xpool, tc.tile_pool(name="e", bufs=6) as epool, tc.tile_pool(
            name="r", bufs=2
        ) as rpool, tc.tile_pool(name="o", bufs=3) as opool, tc.tile_pool(
            name="pss", bufs=2, space="PSUM"
        ) as pss, tc.tile_pool(
            name="pso", bufs=2, space="PSUM"
        ) as pso, tc.tile_pool(name="pj", bufs=2, space="PSUM") as pjp:
            # ---- DMAs split across the two hardware DGE rings (sync +
            # scalar); the first chunk's X and Wq are split per-kc so the
            # first projection matmuls can start as soon as each 128-row
            # slice lands (Tile tracks subtile deps) ----
            bq_sb = cpool.tile([DH, NH], F32, name="bq_sb")
            nc.sync.dma_start(bq_sb[:], bq[:])

            wq_sb = wpool.tile([P, KC, FG], BF16, name="wq_sb")
            wk_sb = wpool.tile([P, KC, FG], BF16, name="wk_sb")
            wv_sb = wpool.tile([P, KC, FG], BF16, name="wv_sb")

            xts = [None] * NQC

            def fetch_x(c):
                xts[c] = xpool.tile([P, KC, TQ], BF16, tag="xt", name="xt_t")
                if c == 0:
                    for kc in range(KC):
                        nc.sync.dma_start(
                            xts[c][:, kc, :],
                            xt[kc * P : (kc + 1) * P, 0:TQ],
                        )
                        nc.scalar.dma_start(
                            wq_sb[:, kc, :], wq[kc * P : (kc + 1) * P, :]
                        )
                else:
                    nc.sync.dma_start(
                        xts[c][:],
                        xt[:, c * TQ : (c + 1) * TQ].rearrange(
                            "(kc p) t -> p kc t", p=P
                        ),
                    )

            fetch_x(0)
            nc.scalar.dma_start(
                wk_sb[:], wk.rearrange("(kc p) m -> p kc m", p=P)
            )
            nc.sync.dma_start(
                wv_sb[:], wv.rearrange("(kc p) m -> p kc m", p=P)
            )
            cm_sb = cpool.tile([P, 4, TQ], BF16, name="cm_sb")
            nc.scalar.dma_start(cm_sb[:], cm[:])

            qt_sb = qkv.tile([DH, NH, S], BF16, name="qt_sb")
            kt_sb = qkv.tile([DH, NH, S], BF16, name="kt_sb")
            v_sb = qkv.tile([P, NKC, NH, 2 * DH], BF16, name="v_sb")
            u_sb = qkv.tile([P, FC, S], BF16, name="u_sb")

            wo_sb = wpool.tile([P, FC, D], BF16, name="wo_sb")

            # ---- projection emission for one token chunk, split in units ----
            # unit list interleaves Q/K/V so each is spread over the chunk
            UNITS = [
                ("q", 0), ("k", 0), ("v", 0),
                ("q", 1), ("k", 1), ("v", 1),
                ("q", 2), ("k", 2), ("v", 2),
                ("q", 3), ("k", 3), ("v", 3),
            ]

            def emit_unit(c, u):
                kind, idx = UNITS[u]
                tsl = slice(c * TQ, (c + 1) * TQ)
                if kind in ("q", "k"):
                    w_sb = wq_sb if kind == "q" else wk_sb
                    dst = qt_sb if kind == "q" else kt_sb
                    ps = pjp.tile([P, TQ], F32, tag="pj", name="ps_qk")
                    for kc in range(KC):
                        nc.tensor.matmul(
                            ps[:],
                            w_sb[:, kc, idx * P : (idx + 1) * P],
                            xts[c][:, kc, :],
                            start=(kc == 0),
                            stop=(kc == KC - 1),
                        )
                    for hh in range(2):
                        hx = 2 * idx + hh
                        if kind == "q":
                            nc.vector.tensor_tensor(
                                dst[:, hx, tsl],
                                ps[hh * DH : (hh + 1) * DH, :],
                                bq_sb[:, hx : hx + 1].to_broadcast([DH, TQ]),
                                mybir.AluOpType.add,
                            )
                        else:
                            nc.vector.tensor_copy(
                                dst[:, hx, tsl], ps[hh * DH : (hh + 1) * DH, :]
                            )
                else:
                    ps = pjp.tile([P, NH, DH], F32, tag="pj", name="ps_v")
                    for kc in range(KC):
                        nc.tensor.matmul(
                            ps[:],
                            xts[c][:, kc, idx * P : (idx + 1) * P],
                            wv_sb[:, kc, :],
                            start=(kc == 0),
                            stop=(kc == KC - 1),
                        )
                    tc_idx = c * (TQ // P) + idx
                    nc.vector.tensor_copy(v_sb[:, tc_idx, :, DH : 2 * DH], ps[:])

            # ---- attention for one (head, q-chunk) ----
            def emit_av(po, h, e_t, pr, kind, stop):
                kc0 = 2 * pr
                c0s = (256, 384) if kind == 2 else (0, 128) if kind == 1 else (0, 0)
                for s in range(2):
                    c0 = c0s[s]
                    nc.tensor.matmul(
                        po[:, c0:],
                        v_sb[:, kc0 + s, h, :],
                        e_t[:, s, c0:],
                        start=(pr == 0 and s == 0),
                        stop=(stop and s == 1),
                        skip_group_check=True,
                    )

            # attn@V matmuls trail their scores by AV_LAG pairs, carried in a
            # queue that crosses head boundaries: the next head's scores and
            # the boundary filler matmuls cover the exp+mask latency instead
            # of the tensor engine stalling at every head's first pair
            AV_LAG = 2
            avq = []

            def flush_av():
                po, h, qc, e_t, pr, kind, last = avq.pop(0)
                emit_av(po, h, e_t, pr, kind, stop=last)
                if last:
                    hi, hp = h // 2, h % 2
                    qtsl = slice(qc * TQ, (qc + 1) * TQ)
                    # rows 0-63 of po hold Z replicated (ones block in v_sb)
                    rb = rpool.tile([DH, TQ], F32, tag="rb", name="rb_t")
                    nc.vector.reciprocal_approx_fast(rb[:], po[0:DH, :])
                    nc.vector.tensor_mul(
                        u_sb[hp * DH : (hp + 1) * DH, hi, qtsl],
                        po[DH:P, :],
                        rb[:],
                    )

            def attn(h, qc):
                npairs = (2 * (qc + 1)) if causal else (NKC // 2)
                po = pso.tile([P, TQ], F32, tag="po", name="po")
                for pr in range(npairs):
                    kc0 = 2 * pr
                    kind = 0
                    if causal:
                        if kc0 == 4 * qc:
                            kind = 1
                        elif kc0 == 4 * qc + 2:
                            kind = 2
                    ps = pss.tile([P, 2, TQ], F32, tag="ps", name="ps_s")
                    e_t = epool.tile([P, 2, TQ], BF16, tag="e", name="e_t")
                    c0 = 256 if kind == 2 else 0
                    for s in range(2):
                        nc.tensor.matmul(
                            ps[:, s, c0:],
                            kt_sb[:, h, (kc0 + s) * TK : (kc0 + s + 1) * TK],
                            qt_sb[:, h, qc * TQ + c0 : (qc + 1) * TQ],
                            start=True,
                            stop=True,
                        )
                    nc.scalar.activation(
                        e_t[:, :, c0:], ps[:, :, c0:], AF.Exp, scale=0.125
                    )
                    if kind:
                        # mask each kc block; only columns the attn@V matmul
                        # will actually read need masking
                        mc0s = (256, 384) if kind == 2 else (0, 128)
                        for s in range(2):
                            mc = mc0s[s]
                            j = 2 * (kind - 1) + s
                            nc.vector.tensor_mul(
                                e_t[:, s, mc:],
                                e_t[:, s, mc:],
                                cm_sb[:, j, mc:],
                            )
                    avq.append((po, h, qc, e_t, pr, kind, pr == npairs - 1))
                    while len(avq) > AV_LAG:
                        flush_av()

            # ---- output projection for one 128-token block ----
            def outproj_slice(qc, t8, final=False):
                tg = qc * (TQ // P) + t8
                o_t = opool.tile([P, D], BF16, tag="o", name="o_t")
                for n2 in range(2):
                    ps = pjp.tile([P, 512], F32, tag="pj", name="ps_o")
                    for i in range(FC):
                        nc.tensor.matmul(
                            ps[:],
                            u_sb[:, i, tg * P : (tg + 1) * P],
                            wo_sb[:, i, n2 * 512 : (n2 + 1) * 512],
                            start=(i == 0),
                            stop=(i == FC - 1),
                        )
                    osl = o_t[:, n2 * 512 : (n2 + 1) * 512]
                    if final and n2 == 0:
                        # split the drain-critical evictions across the
                        # then-idle scalar engine and the vector engine
                        nc.scalar.activation(osl, ps[:], AF.Copy)
                    else:
                        nc.vector.tensor_copy(osl, ps[:])
                eng = nc.scalar if final else nc.sync
                eng.dma_start(out[tg * P : (tg + 1) * P, :], o_t[:])

            # ---- schedule ----
            # warm the PE clock (HAM ramps to 2.4GHz after ~4us of continuous
            # matmul activity) with dummy matmuls on a scratch tile while the
            # input DMAs land; they write alternating psum banks, never read.
            # both memsets go on the otherwise-idle gpsimd engine so the
            # vector queue stays clear for projection evictions
            warm_sb = cpool.tile([P, TQ], BF16, name="warm_sb")
            nc.gpsimd.memset(warm_sb[:], 0.0)
            # ones block for the softmax-denominator rows of attn@V
            nc.gpsimd.memset(v_sb[:, :, :, 0:DH], 1.0)
            for _ in range(34):
                wps = pjp.tile([P, TQ], F32, tag="pj", name="ps_warm")
                nc.tensor.matmul(
                    wps[:], warm_sb[:, 0:P], warm_sb[:], start=True, stop=True
                )
            # chunk 0 runs Q then K then V units, matching DMA arrival order
            for kind, idx in [("q", i) for i in range(4)] + [
                ("k", i) for i in range(4)
            ] + [("v", i) for i in range(4)]:
                emit_unit(0, UNITS.index((kind, idx)))
            for qc in range(NQC):
                if qc + 1 < NQC:
                    fetch_x(qc + 1)
                if qc == 0:
                    nc.sync.dma_start(wo_sb[:], wo[:])
                nunits = len(UNITS) if qc + 1 < NQC else 0
                udone = 0
                for h in range(NH):
                    attn(h, qc)
                    uend = nunits * (h + 1) // NH
                    while udone < uend:
                        emit_unit(qc + 1, udone)
                        udone += 1
                    # output projections trail their chunk by two windows so
                    # this filler work lands in the ACT-bound final window
                    if qc == 2 and h % 2 == 1:
                        outproj_slice(0, h // 2)
                    elif qc == 3 and h % 2 == 1:
                        outproj_slice(1, h // 2)
                    elif qc == 3 and h % 2 == 0:
                        outproj_slice(2, h // 2)
                if qc == NQC - 1:
                    # drain the attn@V queue so the final chunk's u is
                    # emitted (and thus dep-tracked) before its consumers
                    while avq:
                        flush_av()
                    for t8 in range(TQ // P):
                        outproj_slice(qc, t8, final=True)

    nc.compile()
    return nc


def make_in_maps(X, Wq, bq, Wk, Wv, Wo, causal):
    bf = ml_dtypes.bfloat16
    # mask[p, j, f] = 1.0 where k-position p of diagonal chunk j may attend
    # to q-position f of the 512-wide q tile: p <= f - 128*j
    pv = np.arange(P)[:, None, None]
    jv = np.arange(4)[None, :, None]
    fv = np.arange(TQ)[None, None, :]
    cmv = (pv <= fv - TK * jv).astype(bf)
    in_maps = []
    for b in range(4):
        for g in range(2):
            sl = slice(g * FG, (g + 1) * FG)
            in_maps.append(
                {
                    "XT": np.ascontiguousarray(X[b].T).astype(bf),
                    "WQ": np.ascontiguousarray(Wq[:, sl]).astype(bf),
                    "WK": np.ascontiguousarray(Wk[:, sl]).astype(bf),
                    "WV": np.ascontiguousarray(Wv[:, sl]).astype(bf),
                    "BQ": np.ascontiguousarray(bq[sl].reshape(NH, DH).T).astype(
                        np.float32
                    ),
                    "WO": np.ascontiguousarray(
                        Wo[sl, :].reshape(FC, P, D).transpose(1, 0, 2)
                    ).astype(bf),
                    "CM": cmv,
                }
            )
    return in_maps


_CACHE = {}


def _get_program(causal):
    key = bool(causal)
    if key not in _CACHE:
        _CACHE[key] = build(tokens=2048, causal=key)
    return _CACHE[key]


def kernel(X, Wq, bq, Wk, bk, Wv, bv, Wo, bo, causal, **_unused):
    from concourse.bass_utils import run_bass_kernel_spmd

    X = np.asarray(X, np.float32)
    Wq, bq = np.asarray(Wq, np.float32), np.asarray(bq, np.float32)
    Wk = np.asarray(Wk, np.float32)
    Wv = np.asarray(Wv, np.float32)
    Wo, bo = np.asarray(Wo, np.float32), np.asarray(bo, np.float32)
    bv = np.asarray(bv, np.float32)
    causal_flag = bool(np.asarray(causal).item())

    nc = _get_program(causal_flag)
    in_maps = make_in_maps(X, Wq, bq, Wk, Wv, Wo, causal_flag)
    res = run_bass_kernel_spmd(nc, in_maps, core_ids=list(range(8)))

    # attn rows sum to 1, so the missing V bias contributes bv @ Wo exactly
    corr = bv @ Wo + bo
    outs = []
    for b in range(4):
        o = (
            res.results[2 * b]["OUT"].astype(np.float32)
            + res.results[2 * b + 1]["OUT"].astype(np.float32)
            + corr
        )
        outs.append(o)
    return np.stack(outs).astype(np.float32)
